# revision 13
# baseline (speedup 1.0000x reference)
"""2-layer GCN (GCNConv x2) on 8 Trainium2 NeuronCores.

Strategy (dst-sharded, edge-partitioned by destination):
- Each core owns N/8 destination nodes and the edges pointing at them.
- h~ = dinv * (x @ W1) computed per-shard, AllGathered to a full bf16 table.
- Per-edge messages fetched with dma_gather (4 SWDGE queues round-robin);
  scatter-add done as one-hot-indicator matmuls accumulating in PSUM
  (indicator = is_equal(iota, dstloc) * dinv[dst], built on DVE per chunk).
- Layer 1 accumulates transposed (aggT [hid, dst]) so bias+ReLU ride the
  activation engine per-partition and the block's h2 = out1 @ W2 matmul can
  consume it directly as lhsT; h2~ = dinv * h2 written f32, AllGathered,
  layer 2 repeats the same edge schedule against the h2 table.
"""
import sys
import types

import numpy as np
import ml_dtypes

P = 128
NCORES = 8
GMAX_CHUNKS = 32  # max chunks (128 idxs each) per dma_gather
SB_N = 6  # dst blocks per super-block (one PSUM bank each; 6+1+1 banks)
NQUEUES = 4

_CACHE = {}


# ---------------------------------------------------------------- compat ---
def _install_compat():
    """Patches for this axon/walrus stack (drain waits, per-inst wait caps,
    NTFF shim). Idempotent."""
    if _CACHE.get("compat"):
        return
    import concourse.tile as tile
    import concourse.mybir as mybir

    _ev = [0]

    def _split_inst_waits(ordered):
        for _bb, insts in ordered.items():
            out = []
            for inst in insts:
                si = getattr(inst, "sync_info", None)
                if si is not None and si.on_wait is not None and len(si.on_wait) > 1:
                    waits = list(si.on_wait)
                    excess, keep = waits[:-1], waits[-1:]
                    si.on_wait.clear()
                    for sw in keep:
                        si.on_wait.append(sw)
                    for i in range(0, len(excess), 2):
                        _ev[0] += 1
                        ev = mybir.InstEventSemaphore(
                            name=f"evsplit-{_ev[0]}", ins=[], outs=[]
                        )
                        ev.engine = inst.engine
                        ev.sync_info = mybir.SyncInfo(
                            on_wait=excess[i : i + 2], on_update=[]
                        )
                        out.append(ev)
                out.append(inst)
            insts[:] = out

    orig_lower = tile.TileContext._lower_ordered_insts

    def patched_lower(self, ordered):
        _split_inst_waits(ordered)
        return orig_lower(self, ordered)

    def patched_drain(self, tick_clock, wait_clock):
        sems_alloc = list(self.sems.allocated().values())
        carrier = self.nc.sync.wait_ge(sems_alloc[0], 0)
        wait_clock.add_sem_waits(
            carrier.ins, tile.ScopedClock({None: tick_clock.global_clock})
        )
        waits = list(carrier.ins.sync_info.on_wait)
        carrier.ins.sync_info.on_wait.clear()
        for sw in waits[:2]:
            carrier.ins.sync_info.on_wait.append(sw)
        for i in range(2, len(waits), 2):
            c = self.nc.sync.wait_ge(sems_alloc[0], 0)
            c.ins.sync_info.on_wait.clear()
            for sw in waits[i : i + 2]:
                c.ins.sync_info.on_wait.append(sw)
        self.nc.sync.drain(fusable=False)
        self.nc.all_engine_barrier()
        popped = self.nc._tile_sem_poison_stack.pop()
        assert popped is self._sem_poison
        self.nc.clear_and_free_semaphores(sems_alloc)
        self.nc.all_engine_barrier()

    tile.TileContext._lower_ordered_insts = patched_lower
    tile.TileContext._drain_and_barrier = patched_drain

    # NTFF profile hook shim (missing antenv.axon_hooks in this image)
    _hook = {}
    mod = types.ModuleType("antenv.axon_hooks")
    mod.set_axon_ntff_profile_hook = lambda h: _hook.update(hook=h)
    mod.get_axon_ntff_profile_hook = lambda: _hook.get("hook")
    sys.modules["antenv.axon_hooks"] = mod
    try:
        import antenv

        antenv.axon_hooks = mod
        from trn_agent_boot.trn_boot import _ntff_profile_via_ctypes

        mod.set_axon_ntff_profile_hook(
            _ntff_profile_via_ctypes("/opt/axon/libaxon_pjrt.so")
        )
    except Exception:
        pass
    _CACHE["compat"] = True


# ---------------------------------------------------------- preprocessing ---
class Schedule:
    pass


def _preprocess(n, edge_index):
    """Build the uniform cross-core schedule + per-core data streams."""
    shard = n // NCORES
    nblk = (shard + P - 1) // P
    nbanks = 4
    bank_rows = (n + nbanks - 1) // nbanks
    assert bank_rows <= 32767
    n_sb = (nblk + SB_N - 1) // SB_N

    src = edge_index[0].astype(np.int64)
    dst = edge_index[1].astype(np.int64)
    e = src.shape[0]
    deg = np.bincount(dst, minlength=n).astype(np.float64) + 1.0
    dinv = (1.0 / np.sqrt(deg)).astype(np.float32)

    # append self loops
    loops = np.arange(n, dtype=np.int64)
    src2 = np.concatenate([src, loops])
    dst2 = np.concatenate([dst, loops])

    core = dst2 // shard
    dl = dst2 - core * shard
    blk = dl // P
    dstloc = (dl % P).astype(np.int32)
    bank = src2 // bank_rows
    bidx = (src2 % bank_rows).astype(np.int32)

    # per-core counts per (block, bank)
    cnt = np.zeros((NCORES, nblk, nbanks), np.int64)
    flat = (core * nblk + blk) * nbanks + bank
    bc = np.bincount(flat, minlength=NCORES * nblk * nbanks)
    cnt[...] = bc.reshape(NCORES, nblk, nbanks)
    budget = np.ceil(cnt.max(axis=0) / P).astype(np.int64)  # [nblk, nbanks] chunks

    # schedule: for sb -> for bank -> for blk in sb (budget>0): chunks
    chunk_block = []  # global chunk idx -> block
    chunk_start = []
    chunk_stop = []
    gathers = []  # (col16_off, num_idxs, bank, chunk_off)
    seen_first = np.zeros(nblk, bool)
    # total chunks per block to detect last
    blk_total = budget.sum(axis=1)
    blk_done = np.zeros(nblk, np.int64)
    slot_off = 0
    sb_post = []  # per sb: list of blocks
    for s in range(n_sb):
        blocks = list(range(s * SB_N, min((s + 1) * SB_N, nblk)))
        for k in range(nbanks):
            seg = []  # (block, nchunks)
            for b in blocks:
                if budget[b, k] > 0:
                    seg.append((b, int(budget[b, k])))
            tot = sum(x[1] for x in seg)
            # split into gathers
            coff = len(chunk_block)
            for b, nch in seg:
                for j in range(nch):
                    chunk_block.append(b)
                    chunk_start.append(not seen_first[b])
                    seen_first[b] = True
                    blk_done[b] += 1
                    chunk_stop.append(blk_done[b] == blk_total[b])
            g0 = 0
            while g0 < tot:
                gn = min(GMAX_CHUNKS, tot - g0)
                gathers.append(
                    (slot_off // 16, gn * P, k, coff + g0)
                )
                slot_off += gn * P
                g0 += gn
        sb_post.append(blocks)

    totc = len(chunk_block)
    tot_slots = slot_off
    assert tot_slots == totc * P

    # per-core streams
    idx_stream = np.zeros((NCORES, 16, tot_slots // 16), np.int16)
    dstloc_s = np.full((NCORES, P, totc), -1.0, np.float32)
    dinvd_s = np.zeros((NCORES, P, totc), np.float32)

    sb_arr = blk // SB_N
    order = np.lexsort((blk, bank, sb_arr, core))
    so_core = core[order]
    so_blk = blk[order]
    so_bank = bank[order]
    so_bidx = bidx[order]
    so_dstloc = dstloc[order]
    so_dinvd = dinv[dst2[order]]

    # walk schedule per core, consuming sorted runs
    ptr = np.searchsorted(so_core, np.arange(NCORES + 1))
    for c in range(NCORES):
        lo, hi = ptr[c], ptr[c + 1]
        cblk = so_blk[lo:hi]
        cbank = so_bank[lo:hi]
        cbidx = so_bidx[lo:hi]
        cdl = so_dstloc[lo:hi]
        cdv = so_dinvd[lo:hi]
        csb = cblk // SB_N
        # group boundaries: runs of (sb, bank, blk) in this order already
        key = (csb * nbanks + cbank) * nblk + cblk
        # iterate schedule in same order
        pos = 0
        slot = 0
        idx_flat = np.zeros(tot_slots, np.int16)
        dl_flat = np.full(totc * P, -1.0, np.float32)
        dv_flat = np.zeros(totc * P, np.float32)
        for s in range(n_sb):
            blocks = list(range(s * SB_N, min((s + 1) * SB_N, nblk)))
            for k in range(nbanks):
                for b in blocks:
                    bud = int(budget[b, k])
                    if bud == 0:
                        continue
                    want = (s * nbanks + k) * nblk + b
                    cnt_cb = 0
                    while pos + cnt_cb < hi - lo and key[pos + cnt_cb] == want:
                        cnt_cb += 1
                    nsl = bud * P
                    idx_flat[slot : slot + cnt_cb] = cbidx[pos : pos + cnt_cb]
                    dl_flat[slot : slot + cnt_cb] = cdl[pos : pos + cnt_cb]
                    dv_flat[slot : slot + cnt_cb] = cdv[pos : pos + cnt_cb]
                    pos += cnt_cb
                    slot += nsl
        assert pos == hi - lo, (c, pos, hi - lo)
        assert slot == tot_slots
        # wrap: slot i -> idx[i%16, i//16] within each gather's window
        for (c16, nidx, _k, _coff) in gathers:
            sl = slice(c16 * 16, c16 * 16 + nidx)
            seg = idx_flat[sl].reshape(nidx // 16, 16).T  # [16, nidx/16]
            idx_stream[c][:, c16 : c16 + nidx // 16] = seg
        # dstloc layout: chunk C, partition p = slot C*128+p
        dstloc_s[c] = dl_flat.reshape(totc, P).T
        dinvd_s[c] = dv_flat.reshape(totc, P).T

    sch = Schedule()
    sch.n, sch.e, sch.shard, sch.nblk, sch.nbanks = n, e, shard, nblk, nbanks
    sch.bank_rows, sch.n_sb, sch.totc = bank_rows, n_sb, totc
    sch.tot_slots = tot_slots
    sch.chunk_block = chunk_block
    sch.chunk_start = chunk_start
    sch.chunk_stop = chunk_stop
    sch.gathers = gathers
    sch.sb_post = sb_post
    sch.budget = budget
    sch.dinv = dinv
    sch.idx_stream = np.tile(idx_stream, (1, 8, 1))  # replicate to 128 partitions
    sch.dstloc_s = dstloc_s
    sch.dinvd_s = dinvd_s
    return sch


# ----------------------------------------------------------------- build ---
def _build(sch, in_dim, hid, out_dim):
    import concourse.mybir as mybir
    import concourse.tile as tile
    from concourse import bacc

    bf16 = mybir.dt.bfloat16
    f32 = mybir.dt.float32
    shard, nblk, nbanks = sch.shard, sch.nblk, sch.nbanks
    totc, n_sb = sch.totc, sch.n_sb
    n = sch.n

    nc = bacc.Bacc(num_swdge_queues=NQUEUES)

    xT = nc.declare_dram_parameter("xT", [in_dim, shard], bf16, isOutput=False)
    idxs = nc.declare_dram_parameter(
        "idxs", [P, sch.tot_slots // 16], mybir.dt.int16, isOutput=False
    )
    dstloc = nc.declare_dram_parameter("dstloc", [P, totc], f32, isOutput=False)
    dinvd = nc.declare_dram_parameter("dinvd", [P, totc], f32, isOutput=False)
    dinvb = nc.declare_dram_parameter("dinvb", [P, nblk], f32, isOutput=False)
    w1 = nc.declare_dram_parameter("W1", [in_dim, hid], bf16, isOutput=False)
    b1 = nc.declare_dram_parameter("b1", [hid, 1], f32, isOutput=False)
    w2 = nc.declare_dram_parameter("W2", [hid, out_dim], bf16, isOutput=False)
    b2bc = nc.declare_dram_parameter("b2bc", [P, out_dim], f32, isOutput=False)
    iota_in = nc.declare_dram_parameter("iota", [P, P], bf16, isOutput=False)
    out_ext = nc.declare_dram_parameter("out", [shard, out_dim], f32, isOutput=True)

    hloc = nc.dram_tensor("hloc", [shard, hid], bf16)
    hfull = nc.dram_tensor("hfull", [n, hid], bf16, addr_space="Shared")
    h2loc = nc.dram_tensor("h2loc", [shard, out_dim], f32)
    h2full = nc.dram_tensor("h2full", [n, out_dim], f32, addr_space="Shared")

    kin = in_dim // P  # contraction tiles for layer-1 matmul

    with tile.TileContext(nc) as tc:
        with (
            tc.tile_pool(name="const", bufs=1) as cpool,
            tc.tile_pool(name="xload", bufs=2) as xpool,
            tc.tile_pool(name="hb", bufs=2) as hbpool,
            tc.tile_pool(name="idx", bufs=4) as ipool,
            tc.tile_pool(name="gath", bufs=4) as gpool,
            tc.tile_pool(name="sind", bufs=8) as spool,
            tc.tile_pool(name="conv", bufs=8) as vpool,
            tc.tile_pool(name="blk", bufs=3) as bpool,
            tc.tile_pool(name="psh", bufs=1, space="PSUM") as psh,
            tc.tile_pool(name="psagg", bufs=6, space="PSUM") as psagg,
            tc.tile_pool(name="psh2", bufs=1, space="PSUM") as psh2,
        ):
            # one register per distinct gather size, set once
            import contextlib

            regstack = contextlib.ExitStack()
            nidx_vals = sorted({g[1] for g in sch.gathers})
            nreg_map = {}
            for v in nidx_vals:
                r = regstack.enter_context(nc.gpsimd.register(f"nreg_{v}"))
                nc.gpsimd.reg_mov(r, v)
                nreg_map[v] = r
            # ---- constants into SBUF
            iota_sb = cpool.tile([P, P], bf16, tag="iota")
            nc.sync.dma_start(out=iota_sb[:], in_=iota_in[:])
            w1_t = [cpool.tile([P, hid], bf16, tag=f"w1_{k}", name=f"w1t{k}") for k in range(kin)]
            for k in range(kin):
                nc.sync.dma_start(out=w1_t[k][:], in_=w1[k * P : (k + 1) * P, :])
            w2_sb = cpool.tile([hid, out_dim], bf16, tag="w2")
            nc.sync.dma_start(out=w2_sb[:], in_=w2[:])
            b1_sb = cpool.tile([hid, 1], f32, tag="b1")
            nc.sync.dma_start(out=b1_sb[:], in_=b1[:])
            b2_sb = cpool.tile([P, out_dim], f32, tag="b2")
            nc.sync.dma_start(out=b2_sb[:], in_=b2bc[:])
            dinvb_sb = cpool.tile([P, nblk], f32, tag="dinvb")
            nc.sync.dma_start(out=dinvb_sb[:], in_=dinvb[:])
            dstloc_sb = cpool.tile([P, totc], f32, tag="dstloc")
            nc.sync.dma_start(out=dstloc_sb[:], in_=dstloc[:])
            dinvd_sb = cpool.tile([P, totc], f32, tag="dinvd")
            nc.sync.dma_start(out=dinvd_sb[:], in_=dinvd[:])

            # ---- h~ = dinv * (x @ W1), shard-local, bf16
            XGRP = 8  # blocks of columns per xT load
            for g0 in range(0, nblk, XGRP):
                g1 = min(g0 + XGRP, nblk)
                c0, c1 = g0 * P, min(g1 * P, shard)
                xt = [
                    xpool.tile([P, c1 - c0], bf16, tag=f"xt{k}", name=f"xt{k}")
                    for k in range(kin)
                ]
                for k in range(kin):
                    nc.sync.dma_start(
                        out=xt[k][:], in_=xT[k * P : (k + 1) * P, c0:c1]
                    )
                for b in range(g0, g1):
                    m = min(P, shard - b * P)
                    hp = psh.tile([P, hid], f32, tag="hps")
                    for k in range(kin):
                        nc.tensor.matmul(
                            out=hp[:m, :],
                            lhsT=xt[k][:, b * P - c0 : b * P - c0 + m],
                            rhs=w1_t[k][:],
                            start=(k == 0),
                            stop=(k == kin - 1),
                        )
                    hsb = hbpool.tile([P, hid], bf16, tag="hsb")
                    nc.scalar.activation(
                        out=hsb[:m, :],
                        in_=hp[:m, :],
                        func=mybir.ActivationFunctionType.Copy,
                        scale=dinvb_sb[:m, b : b + 1],
                    )
                    nc.sync.dma_start(
                        out=hloc[b * P : b * P + m, :], in_=hsb[:m, :]
                    )

            nc.gpsimd.collective_compute(
                "AllGather",
                mybir.AluOpType.bypass,
                ins=[hloc[:]],
                outs=[hfull[:]],
                replica_groups=[list(range(NCORES))],
            )

            # ---- layer pipelines
            def run_layer(layer):
                feat = hid if layer == 1 else out_dim
                table = hfull if layer == 1 else h2full
                tdt = bf16 if layer == 1 else f32
                gq = [0]
                for s in range(n_sb):
                    blocks = sch.sb_post[s]
                    w = P if layer == 1 else out_dim
                    agg_t = {
                        b: psagg.tile([P, w], f32, tag="agg", name=f"agg{s}_{b}")
                        for b in blocks
                    }

                    def slot(b):
                        return agg_t[b][:, :]

                    blocks_set = set(blocks)
                    for (c16, nidx, k, coff) in [
                        g
                        for g in sch.gathers
                        if sch.chunk_block[g[3]] in blocks_set
                    ]:
                        nch = nidx // P
                        it = ipool.tile([P, GMAX_CHUNKS * 8], mybir.dt.int16, tag="it")
                        nc.sync.dma_start(
                            out=it[:, : nidx // 16],
                            in_=idxs[:, c16 : c16 + nidx // 16],
                        )
                        gt = gpool.tile([P, GMAX_CHUNKS, feat], tdt, tag="gt")
                        r0 = k * sch.bank_rows
                        r1 = min(r0 + sch.bank_rows, n)
                        nc.gpsimd.dma_gather(
                            out_ap=gt[:, :nch, :],
                            in_ap=table[r0:r1, :],
                            idxs_ap=it[:, : nidx // 16],
                            num_idxs=nidx,
                            num_idxs_reg=nreg_map[nidx],
                            elem_size=feat,
                            single_packet=False,
                            queue_num=gq[0] % NQUEUES,
                        )
                        gq[0] += 1
                        for j in range(nch):
                            C = coff + j
                            b = sch.chunk_block[C]
                            st = sch.chunk_start[C]
                            sp = sch.chunk_stop[C]
                            sind = spool.tile([P, P], bf16, tag="sind")
                            nc.vector.tensor_scalar(
                                sind[:],
                                iota_sb[:],
                                dstloc_sb[:, C : C + 1],
                                dinvd_sb[:, C : C + 1],
                                mybir.AluOpType.is_equal,
                                mybir.AluOpType.mult,
                            )
                            if layer == 1:
                                nc.tensor.matmul(
                                    out=slot(b),
                                    lhsT=gt[:, j, :],
                                    rhs=sind[:],
                                    start=st,
                                    stop=sp,
                                )
                            else:
                                cv = vpool.tile([P, out_dim], bf16, tag="cv")
                                nc.vector.tensor_copy(
                                    out=cv[:], in_=gt[:, j, :]
                                )
                                nc.tensor.matmul(
                                    out=slot(b),
                                    lhsT=sind[:],
                                    rhs=cv[:],
                                    start=st,
                                    stop=sp,
                                )
                    # ---- block epilogue for this super-block
                    for b in blocks:
                        m = min(P, shard - b * P)
                        if layer == 1:
                            o1 = bpool.tile([P, P], bf16, tag="o1")
                            nc.scalar.activation(
                                out=o1[:],
                                in_=slot(b),
                                func=mybir.ActivationFunctionType.Relu,
                                bias=b1_sb[:, :1],
                            )
                            h2p = psh2.tile([P, out_dim], f32, tag="h2p")
                            nc.tensor.matmul(
                                out=h2p[:],
                                lhsT=o1[:],
                                rhs=w2_sb[:],
                                start=True,
                                stop=True,
                            )
                            h2s = bpool.tile([P, out_dim], f32, tag="h2s")
                            nc.scalar.activation(
                                out=h2s[:m, :],
                                in_=h2p[:m, :],
                                func=mybir.ActivationFunctionType.Copy,
                                scale=dinvb_sb[:m, b : b + 1],
                            )
                            nc.sync.dma_start(
                                out=h2loc[b * P : b * P + m, :], in_=h2s[:m, :]
                            )
                        else:
                            ob = bpool.tile([P, out_dim], f32, tag="ob")
                            nc.vector.tensor_tensor(
                                out=ob[:m, :],
                                in0=slot(b)[:m, :],
                                in1=b2_sb[:m, :],
                                op=mybir.AluOpType.add,
                            )
                            nc.sync.dma_start(
                                out=out_ext[b * P : b * P + m, :], in_=ob[:m, :]
                            )

            run_layer(1)
            nc.gpsimd.collective_compute(
                "AllGather",
                mybir.AluOpType.bypass,
                ins=[h2loc[:]],
                outs=[h2full[:]],
                replica_groups=[list(range(NCORES))],
            )
            run_layer(2)
            regstack.close()

    nc.compile()
    return nc


# ---------------------------------------------------------------- kernel ---
def _make_in_maps(sch, x, W1, b1v, W2, b2v):
    hid = W1.shape[1]
    out_dim = W2.shape[1]
    shard = sch.shard
    bf = ml_dtypes.bfloat16
    in_maps = []
    w1b = W1.astype(bf)
    w2b = W2.astype(bf)
    b1c = b1v.reshape(hid, 1).astype(np.float32).copy()
    b2c = np.broadcast_to(b2v.astype(np.float32), (P, out_dim)).copy()
    iota = np.broadcast_to(np.arange(P, dtype=np.float32), (P, P)).astype(bf)
    for c in range(NCORES):
        xs = np.ascontiguousarray(x[c * shard : (c + 1) * shard].astype(bf).T)
        dv = sch.dinv[c * shard : (c + 1) * shard]
        full = np.zeros(sch.nblk * P, np.float32)
        full[:shard] = dv
        dvb = np.ascontiguousarray(full.reshape(sch.nblk, P).T)
        in_maps.append(
            {
                "xT": xs,
                "idxs": sch.idx_stream[c],
                "dstloc": sch.dstloc_s[c],
                "dinvd": sch.dinvd_s[c],
                "dinvb": dvb,
                "W1": w1b,
                "b1": b1c,
                "W2": w2b,
                "b2bc": b2c,
                "iota": iota,
            }
        )
    return in_maps


def _get_compiled(n, e, edge_index, in_dim, hid, out_dim):
    key = ("nc", n, e)
    if key not in _CACHE:
        sch = _preprocess(n, edge_index)
        _CACHE[("sched", n, e)] = sch
        _CACHE[key] = _build(sch, in_dim, hid, out_dim)
    return _CACHE[("sched", n, e)], _CACHE[key]


def kernel(x, edge_index, W1, b1, W2, b2):
    _install_compat()
    from concourse.bass_utils import run_bass_kernel_spmd

    x = np.asarray(x)
    edge_index = np.asarray(edge_index)
    W1 = np.asarray(W1, np.float32)
    b1v = np.asarray(b1, np.float32)
    W2 = np.asarray(W2, np.float32)
    b2v = np.asarray(b2, np.float32)
    n, in_dim = x.shape
    hid = W1.shape[1]
    out_dim = W2.shape[1]

    sch, nc = _get_compiled(n, edge_index.shape[1], edge_index, in_dim, hid, out_dim)
    in_maps = _make_in_maps(sch, x, W1, b1v, W2, b2v)
    import os

    trace = bool(os.environ.get("GCN_TRACE"))
    res = run_bass_kernel_spmd(
        nc, in_maps, core_ids=list(range(NCORES)), trace=trace
    )
    global LAST_EXEC_NS
    LAST_EXEC_NS = res.exec_time_ns
    return np.concatenate([res.results[c]["out"] for c in range(NCORES)], axis=0)


LAST_EXEC_NS = None
